# revision 55
# baseline (speedup 1.0000x reference)
"""CHESHIRE hyperedge link predictor on 8 Trainium2 NeuronCores.

Structure exploited (verified at runtime):
  - members[e] = base[e] + arange(8): each hyperedge is a contiguous
    8-node window -> sorting hyperedges by base makes the whole problem
    embarrassingly parallel across a node-range partition (no collectives).
  - edge_index is the full directed 8-clique per hyperedge -> deg == 7,
    w == -1/7, so the K=3 ChebConv folds into a per-entry GEMM plus a
    per-window GEMM:  u_i = (r * x_i) @ Wap + (r * S) @ Wd + D0
    with r = 1/sqrt(var+eps) (GraphNorm fold), S = window sum of x.
  - clip commutes with max/min pools; clip(u)^2 == min(u^2, 1) for the rms
    pool, so hardtanh is applied to pooled maxima and to u^2.

Implementation notes (v3):
  - pos / W_enc cast to bf16 on host (halves the HBM stream; full-rate PE).
  - encoder channel-major (512-wide matmuls), PE transposes to rows, DRAM
    roundtrip for the SWDGE transpose-gather (as v1).
  - hardtanh applied to the *gathered* xe (DVE tensor_scalar 4x mode)
    instead of per-node, freeing phase A.
  - all phase-B elementwise bf16 packed SBUF (DVE 2x), strided mega-ops.
  - Q = sum(x^2) and ssq = sum(min(u^2,1)) on the PE (identity-matmul
    accumulation); D0 folded into the ACT psum evacuation bias.
  - only Sqrt/Square/Identity ACT functions (single table set); sigmoid on
    host; 1/sqrt via DVE reciprocal_approx_fast (accuracy).
  - GraphNorm variance chain on GPSIMD (idle otherwise).
"""

import math

import numpy as np

N_CORES = 8
M = 8          # nodes per hyperedge
D = 128        # embedding dim
F = 512        # input feature dim
EPS = 1e-5
GSZ = 512      # windows per chunk (one f32 PSUM bank per entry)

_GRAPH_CACHE = {}
LAST_EXEC_NS = None
LAST_RESULT = None


def _bf16_dtype():
    import ml_dtypes

    return np.dtype(ml_dtypes.bfloat16)


def _fold_weights(W_enc, b_enc, gn_gamma, gn_beta, gn_alpha, cheb_W, cheb_b,
                  lin_W, lin_b):
    f32 = np.float32
    W0, W1, W2 = (np.asarray(cheb_W[i], f32) for i in range(3))
    gam = np.asarray(gn_gamma, f32)
    bet = np.asarray(gn_beta, f32)
    alp = np.asarray(gn_alpha, f32)
    Wa = W0 + W1 / f32(7.0) - f32(47.0 / 49.0) * W2
    Wb = -W1 / f32(7.0) + f32(12.0 / 49.0) * W2
    Wap = gam[:, None] * Wa
    Wd = -(gam * alp / f32(8.0))[:, None] * (Wa + f32(8.0) * Wb) + gam[:, None] * Wb
    D0 = bet @ Wa + f32(8.0) * (bet @ Wb) + np.asarray(cheb_b, f32)
    neg_ka8 = -(f32(2.0) * alp - alp * alp) / f32(8.0)
    bf16 = _bf16_dtype()
    return {
        "wenc": np.ascontiguousarray(
            np.asarray(W_enc, f32).reshape(4, 128, 128)
            .transpose(1, 0, 2)).astype(bf16),
        "wap": np.ascontiguousarray(Wap.astype(bf16)),
        "wd": np.ascontiguousarray(Wd.astype(bf16)),
        "d0": np.ascontiguousarray(D0.reshape(128, 1)),
        "nka8": np.ascontiguousarray(neg_ka8.reshape(128, 1)),
        "ident": np.eye(128, dtype=bf16),
    }


def _build_graph(u_pad, e_pad, gdeps=None):
    """Build the per-core Bass graph. SPMD: same graph on all 8 cores."""
    import concourse.bass as bass
    import concourse.tile as tile
    from concourse import bacc, mybir

    f32 = mybir.dt.float32
    bf16 = mybir.dt.bfloat16
    i16 = mybir.dt.int16
    AF = mybir.ActivationFunctionType
    OP = mybir.AluOpType

    nb = u_pad // 512          # encoder blocks (13)
    ng = e_pad // GSZ          # window chunks
    ns16 = e_pad // 16

    nc = bacc.Bacc()
    # posT[b, f, k, n'] = pos[512 b + n', 128 k + f]   (rhs tiles, bf16)
    posT_p = nc.declare_dram_parameter("posT", [nb, 128, 4, 512], bf16, False)
    idx_p = nc.declare_dram_parameter("idx", [128, ns16], i16, False)
    wenc_p = nc.declare_dram_parameter("wenc", [128, 4, 128], bf16, False)
    wap_p = nc.declare_dram_parameter("wap", [128, 128], bf16, False)
    wd_p = nc.declare_dram_parameter("wd", [128, 128], bf16, False)
    d0_p = nc.declare_dram_parameter("d0", [128, 1], f32, False)
    nka8_p = nc.declare_dram_parameter("nka8", [128, 1], f32, False)
    ident_p = nc.declare_dram_parameter("ident", [128, 128], bf16, False)
    # pooled outputs: ymm = clip(max)-clip(min), rms^2*8 = ssq; the tiny
    # [2C -> 1] logits GEMM + sigmoid run on the host
    ymm_p = nc.declare_dram_parameter("ymm", [128, e_pad], bf16, True)
    rms_p = nc.declare_dram_parameter("rms", [128, e_pad], bf16, True)

    # x rows, bf16, padded so overlapping window reads stay in bounds
    x_dram = nc.dram_tensor("x_scratch", [u_pad + M, 128], bf16)

    with tile.TileContext(nc) as tc:
        with (
            tc.tile_pool(name="consts", bufs=1) as consts,
            tc.tile_pool(name="psum_u", bufs=2, space="PSUM") as psum_u,
            tc.tile_pool(name="psum_aux", bufs=3, space="PSUM") as psum_aux,
            tc.tile_pool(name="psum_tr", bufs=1, space="PSUM") as psum_tr,
        ):
            # ---- constants ----
            # wenc (needed by the first matmul) goes first on the scalar
            # queue; everything else rides the otherwise-idle vector/gpsimd
            # queues so the head stays short
            wenc_t = consts.tile([128, 4, 128], bf16)
            nc.scalar.dma_start(out=wenc_t[:, :, :], in_=wenc_p[:, :, :])
            ident_t = consts.tile([128, 128], bf16)
            nc.scalar.dma_start(out=ident_t[:, :], in_=ident_p[:, :])
            wap_t = consts.tile([128, 128], bf16)
            nc.gpsimd.dma_start(out=wap_t[:, :], in_=wap_p[:, :])
            wd_t = consts.tile([128, 128], bf16)
            nc.gpsimd.dma_start(out=wd_t[:, :], in_=wd_p[:, :])
            d0_t = consts.tile([128, 1], f32)
            nc.gpsimd.dma_start(out=d0_t[:, :], in_=d0_p[:, :])
            nka8_t = consts.tile([128, 1], f32)
            nc.gpsimd.dma_start(out=nka8_t[:, :], in_=nka8_p[:, :])
            idx_t = consts.tile([128, ns16], i16)
            nc.gpsimd.dma_start(out=idx_t[:, :], in_=idx_p[:, :])
            eps_t = consts.tile([128, 1], f32)
            nc.vector.memset(eps_t[:, :], EPS)
            from concourse import library_config
            nc.gpsimd.load_library(library_config.mlp)

            # ---- phase A: x rows = hardtanh(pos @ W_enc) ----
            # two passes so the PE sees long uninterrupted matmul streams
            # (p-state ramps to full clock): (1) all encoder GEMM blocks,
            # (2) all transposes.  The hardtanh is folded into the DVE
            # psum evacuation.
            x_writes = []
            with (
                tc.tile_pool(name="pos", bufs=3) as pos_pool,
                tc.tile_pool(name="stage", bufs=3) as stage_pool,
            ):
                xT_all = consts.tile([128, u_pad], bf16)
                for b in range(nb):
                    pos_tile = pos_pool.tile([128, 4, 512], bf16, tag="pos")
                    nc.sync.dma_start(out=pos_tile[:, :, :],
                                      in_=posT_p[b, :, :, :])
                    # encoder borrows the (idle) u-GEMM psum ring
                    psu = psum_u.tile([128, 2, GSZ], f32, tag="u")
                    ps = psu[:, 0, :]
                    for k in range(4):
                        nc.tensor.matmul(
                            ps,
                            lhsT=wenc_t[:, k, :],
                            rhs=pos_tile[:, k, :],
                            start=(k == 0),
                            stop=(k == 3),
                        )
                    nc.vector.tensor_scalar(
                        out=xT_all[:, b * 512:(b + 1) * 512], in0=ps,
                        scalar1=1.0, scalar2=-1.0, op0=OP.min, op1=OP.max)
                for b in range(nb):
                    # two alternating half-bank tiles so consecutive blocks
                    # of transposes pipeline against the stage copies
                    pst = psum_tr.tile([128, 4, 128], bf16, tag="tr")
                    for j in range(4):
                        nc.tensor.transpose(
                            out=pst[:, j, :],
                            in_=xT_all[:, b * 512 + j * 128:
                                       b * 512 + (j + 1) * 128],
                            identity=ident_t[:, :],
                        )
                    stg = stage_pool.tile([128, 4, 128], bf16, tag="stage")
                    nc.vector.tensor_copy(out=stg[:, :, :], in_=pst[:, :, :])
                    out_ap = bass.AP(
                        tensor=x_dram, offset=b * 512 * 128,
                        ap=[[128, 128], [128 * 128, 4], [1, 128]])
                    # scalar queue: the sync queue is saturated with pos
                    # loads, and gathers block on these writes
                    w = nc.scalar.dma_start(out=out_ap, in_=stg[:, :, :])
                    x_writes.append(w)
                zr = stage_pool.tile([128, 128], bf16, tag="zpad")
                nc.vector.memset(zr[:, :], 0)
                w = nc.gpsimd.dma_start(out=x_dram[u_pad:u_pad + M, :],
                                        in_=zr[0:M, :])
                x_writes.append(w)

            with (
                tc.tile_pool(name="xe", bufs=4) as xe_pool,
                tc.tile_pool(name="work", bufs=3) as work_pool,
                tc.tile_pool(name="stats", bufs=2) as stats_pool,
            ):
                # ---- phase B: gather windows (transposed), per chunk ----
                nsc = GSZ // 16
                x_view = bass.AP(tensor=x_dram, offset=0,
                                 ap=[[128, u_pad], [1, M * 128]])
                xes = []
                for c in range(ng):
                    xeT = xe_pool.tile([128, M, GSZ], bf16, tag="xe")
                    xes.append(xeT)
                    g = nc.gpsimd.dma_gather(
                        out_ap=xeT[:, :, :],
                        in_ap=x_view,
                        idxs_ap=idx_t[:, c * nsc:(c + 1) * nsc],
                        num_idxs=GSZ,
                        num_idxs_reg=GSZ,
                        elem_size=M * 128,
                        elem_step=128,
                        transpose=True,
                    )
                    if gdeps is None:
                        deps = x_writes
                    else:
                        deps = x_writes[:gdeps[c]] + [x_writes[-1]]
                    for w in deps:
                        tile.add_dep_helper(g.ins, w.ins, reason="x_dram RAW")

                ymm_all = consts.tile([128, e_pad], bf16)
                rms_all = consts.tile([128, e_pad], bf16)
                for c in range(ng):
                    cs = slice(c * GSZ, (c + 1) * GSZ)
                    xeT = xes[c]
                    # psum ring slot for ssq reserved FIRST: it is freed
                    # late (rms evac) and used late, while Q/S slots free
                    # early -- this alignment keeps chunk c+1's early
                    # matmuls off chunk c's critical tail
                    sps = psum_aux.tile([128, GSZ], f32, tag="aux")
                    # x^2: half on ACT, half on DVE
                    sq = work_pool.tile([128, M, GSZ], bf16, tag="sq")
                    nc.scalar.activation(sq[:, 0:4, :], xeT[:, 0:4, :],
                                         AF.Square)
                    nc.vector.tensor_tensor(
                        out=sq[:, 4:8, :], in0=xeT[:, 4:8, :],
                        in1=xeT[:, 4:8, :], op=OP.mult)
                    # Q = sum_e x^2 on PE
                    qps = psum_aux.tile([128, GSZ], f32, tag="aux")
                    for e in range(M):
                        nc.tensor.matmul(qps[:, :], lhsT=ident_t[:, :],
                                         rhs=sq[:, e, :], start=(e == 0),
                                         stop=(e == M - 1))
                    Qsb = stats_pool.tile([128, GSZ], f32, tag="Q")
                    nc.scalar.activation(Qsb[:, :], qps[:, :], AF.Identity)
                    # S = sum_e x on PE (exact f32 accumulation: the
                    # v = Q + nka8*S^2 cancellation needs S/Q errors to
                    # correlate, so both come from f32 sums of the same
                    # bf16 entries)
                    ssp = psum_aux.tile([128, GSZ], f32, tag="aux")
                    for e in range(M):
                        nc.tensor.matmul(ssp[0:128, :], lhsT=ident_t[:, :],
                                         rhs=xeT[:, e, :], start=(e == 0),
                                         stop=(e == M - 1))
                    S = stats_pool.tile([128, GSZ], f32, tag="S")
                    nc.scalar.activation(S[:, :], ssp[0:128, :], AF.Identity)
                    # v = Q + nka8 * S^2 (STT on DVE, add on GPSIMD)
                    v = stats_pool.tile([128, GSZ], f32, tag="v")
                    nc.vector.scalar_tensor_tensor(
                        out=v[:, :], in0=S[:, :], scalar=nka8_t[:, 0:1],
                        in1=S[:, :], op0=OP.mult, op1=OP.mult)
                    nc.vector.tensor_tensor(
                        out=v[:, :], in0=v[:, :], in1=Qsb[:, :], op=OP.add)
                    # r = 1/sqrt(v/8 + eps): ACT Sqrt + DVE fast reciprocal
                    sv = stats_pool.tile([128, GSZ], f32, tag="sv")
                    nc.scalar.activation(sv[:, :], v[:, :], AF.Sqrt,
                                         bias=eps_t[:, 0:1], scale=0.125)
                    rf = stats_pool.tile([128, GSZ], f32, tag="rf")
                    nc.vector.reciprocal_approx_fast(out=rf[:, :],
                                                     in_=sv[:, :])
                    r = stats_pool.tile([128, GSZ], bf16, tag="r")
                    nc.vector.tensor_copy(out=r[:, :], in_=rf[:, :])
                    p = stats_pool.tile([128, GSZ], bf16, tag="p")
                    nc.vector.tensor_tensor(out=p[:, :], in0=r[:, :],
                                            in1=S[:, :], op=OP.mult)
                    # xs = x * r (broadcast over entries)
                    xs = work_pool.tile([128, M, GSZ], bf16, tag="xs")
                    for e in range(M):
                        nc.vector.tensor_tensor(
                            out=xs[:, e, :], in0=xeT[:, e, :],
                            in1=r[:, :], op=OP.mult)
                    # u GEMM in four groups of 2 entries (PE/ACT pipelined);
                    # D0 folded into the ACT evacuation bias.  sq2 for the
                    # first two groups comes from a second ACT psum read
                    # (Square with the same bias); the last two from DVE.
                    u = work_pool.tile([128, M, GSZ], bf16, tag="u")
                    sq2 = work_pool.tile([128, M, GSZ], bf16, tag="sq2")
                    for g2 in range(4):
                        ups = psum_u.tile([128, 2, GSZ], f32, tag="u")
                        for j in range(2):
                            e = 2 * g2 + j
                            nc.tensor.matmul(ups[:, j, :], lhsT=wap_t[:, :],
                                             rhs=xs[:, e, :], start=True,
                                             stop=False)
                        for j in range(2):
                            nc.tensor.matmul(ups[:, j, :], lhsT=wd_t[:, :],
                                             rhs=p[:, :], start=False,
                                             stop=True)
                        es = slice(2 * g2, 2 * g2 + 2)
                        nc.scalar.activation(u[:, es, :], ups[:, :, :],
                                             AF.Identity, bias=d0_t[:, 0:1],
                                             scale=1.0)
                        if g2 < 2:
                            nc.scalar.activation(sq2[:, es, :], ups[:, :, :],
                                                 AF.Square,
                                                 bias=d0_t[:, 0:1],
                                                 scale=1.0)
                    nc.vector.tensor_tensor(
                        out=sq2[:, 4:8, :], in0=u[:, 4:8, :],
                        in1=u[:, 4:8, :], op=OP.mult)
                    # pools: umax/umin trees (DVE), clip after pooling
                    t1 = work_pool.tile([128, 4, GSZ], bf16, tag="t1")
                    nc.vector.tensor_tensor(
                        out=t1[:, :, :], in0=u[:, 0:4, :], in1=u[:, 4:8, :],
                        op=OP.max)
                    nc.vector.tensor_tensor(
                        out=t1[:, 0:2, :], in0=t1[:, 0:2, :],
                        in1=t1[:, 2:4, :], op=OP.max)
                    umax = stats_pool.tile([128, GSZ], bf16, tag="ux")
                    nc.vector.tensor_tensor(
                        out=umax[:, :], in0=t1[:, 0, :], in1=t1[:, 1, :],
                        op=OP.max)
                    nc.vector.tensor_tensor(
                        out=t1[:, :, :], in0=u[:, 0:4, :], in1=u[:, 4:8, :],
                        op=OP.min)
                    nc.vector.tensor_tensor(
                        out=t1[:, 0:2, :], in0=t1[:, 0:2, :],
                        in1=t1[:, 2:4, :], op=OP.min)
                    umin = stats_pool.tile([128, GSZ], bf16, tag="un")
                    nc.vector.tensor_tensor(
                        out=umin[:, :], in0=t1[:, 0, :], in1=t1[:, 1, :],
                        op=OP.min)
                    nc.vector.tensor_scalar(out=umax[:, :], in0=umax[:, :],
                                            scalar1=1.0, scalar2=-1.0,
                                            op0=OP.min, op1=OP.max)
                    nc.vector.tensor_scalar(out=umin[:, :], in0=umin[:, :],
                                            scalar1=1.0, scalar2=-1.0,
                                            op0=OP.min, op1=OP.max)
                    nc.vector.tensor_tensor(out=ymm_all[:, cs],
                                            in0=umax[:, :],
                                            in1=umin[:, :], op=OP.subtract)
                    # ssq = sum_e min(sq2, 1) on PE
                    nc.vector.tensor_scalar(out=sq2[:, :, :],
                                            in0=sq2[:, :, :],
                                            scalar1=1.0, scalar2=None,
                                            op0=OP.min)
                    for e in range(M):
                        nc.tensor.matmul(sps[:, :], lhsT=ident_t[:, :],
                                         rhs=sq2[:, e, :], start=(e == 0),
                                         stop=(e == M - 1))
                    nc.scalar.activation(rms_all[:, cs], sps[:, :], AF.Sqrt,
                                         scale=0.125)
                nc.gpsimd.dma_start(out=ymm_p[:, :], in_=ymm_all[:, :])
                nc.gpsimd.dma_start(out=rms_p[:, :], in_=rms_all[:, :])

    nc.finalize()
    return nc


def _np_fallback(pos_set, W_enc, b_enc, gn_gamma, gn_beta, gn_alpha, cheb_W,
                 cheb_b, lin_W, lin_b, members, edge_index, batch):
    """Pure-numpy general path (only used if the expected input structure is
    absent; inputs from setup_inputs always take the device path)."""
    f32 = np.float32
    E = members.shape[0]
    num_entries = members.size
    x = np.clip(pos_set @ W_enc + b_enc, -1.0, 1.0).astype(f32)
    xe = x[members.reshape(-1)]
    cnt = np.zeros(E, f32)
    np.add.at(cnt, batch, 1.0)
    mean = np.zeros((E, x.shape[1]), f32)
    np.add.at(mean, batch, xe)
    mean /= cnt[:, None]
    ctr = xe - gn_alpha * mean[batch]
    var = np.zeros((E, x.shape[1]), f32)
    np.add.at(var, batch, ctr * ctr)
    var /= cnt[:, None]
    xe = gn_gamma * ctr / np.sqrt(var + EPS)[batch] + gn_beta
    src, dst = edge_index[0], edge_index[1]
    deg = np.zeros(num_entries, f32)
    np.add.at(deg, dst, 1.0)
    w = -1.0 / np.sqrt(deg[src] * deg[dst])

    def lhat(h):
        out = np.zeros_like(h)
        np.add.at(out, dst, w[:, None] * h[src])
        return out

    tx0 = xe
    tx1 = lhat(tx0)
    out = tx0 @ cheb_W[0] + tx1 @ cheb_W[1]
    tkm1, tkm2 = tx1, tx0
    for k in range(2, cheb_W.shape[0]):
        tk = 2.0 * lhat(tkm1) - tkm2
        out = out + tk @ cheb_W[k]
        tkm1, tkm2 = tk, tkm1
    h = np.clip(out + cheb_b, -1.0, 1.0)
    ymax = np.full((E, h.shape[1]), -np.inf, f32)
    ymin = np.full((E, h.shape[1]), np.inf, f32)
    np.maximum.at(ymax, batch, h)
    np.minimum.at(ymin, batch, h)
    ynorm = np.zeros((E, h.shape[1]), f32)
    np.add.at(ynorm, batch, h * h)
    ynorm = np.sqrt(ynorm / cnt[:, None])
    y = np.concatenate([ymax - ymin, ynorm], axis=1)
    logits = y @ lin_W + lin_b
    return (1.0 / (1.0 + np.exp(-logits))).squeeze(-1).astype(f32)


def _has_window_structure(members, edge_index, batch):
    E, Mm = members.shape
    if Mm != M:
        return False
    base = members[:, 0]
    if not (members == base[:, None] + np.arange(M, dtype=members.dtype)).all():
        return False
    if not (batch == np.repeat(np.arange(E, dtype=batch.dtype), M)).all():
        return False
    row, col = np.where(~np.eye(M, dtype=bool))
    offs = np.arange(E, dtype=np.int64)[:, None] * M
    ei = np.stack([(offs + row[None, :]).ravel(), (offs + col[None, :]).ravel()])
    return (edge_index == ei).all()


def kernel(pos_set, W_enc, b_enc, gn_gamma, gn_beta, gn_alpha, cheb_W, cheb_b,
           lin_W, lin_b, members, edge_index, batch):
    pos_set = np.asarray(pos_set, np.float32)
    members = np.asarray(members)
    edge_index = np.asarray(edge_index)
    batch = np.asarray(batch)
    benc_zero = bool(np.all(np.asarray(b_enc) == 0.0))
    if not _has_window_structure(members, edge_index, batch) or not benc_zero:
        return _np_fallback(
            pos_set, np.asarray(W_enc, np.float32),
            np.asarray(b_enc, np.float32), np.asarray(gn_gamma, np.float32),
            np.asarray(gn_beta, np.float32), np.asarray(gn_alpha, np.float32),
            np.asarray(cheb_W, np.float32), np.asarray(cheb_b, np.float32),
            np.asarray(lin_W, np.float32), np.asarray(lin_b, np.float32),
            members, edge_index, batch)

    N = pos_set.shape[0]
    E = members.shape[0]
    base = members[:, 0].astype(np.int64)
    node_span = (N + N_CORES - 1) // N_CORES                # 6250
    u_pad = ((node_span + M + 511) // 512 + 1) * 512        # 6656 for N=50000
    # quantile split: sort windows by base, give each core an equal count.
    order = np.argsort(base, kind="stable")
    ecnt = (E + N_CORES - 1) // N_CORES
    counts = np.array([min(ecnt, E - c * ecnt) for c in range(N_CORES)])
    offs_pre = np.concatenate([[0], np.cumsum(counts)])
    node_lo = np.zeros(N_CORES, np.int64)
    ok = True
    for c in range(N_CORES):
        ids = order[offs_pre[c]:offs_pre[c + 1]]
        if ids.size == 0:
            node_lo[c] = 0
            continue
        node_lo[c] = base[ids[0]]
        if base[ids[-1]] + M - node_lo[c] > u_pad:
            ok = False
            break
    if not ok:
        core_of = np.minimum(base // node_span, N_CORES - 1)
        order = np.argsort(base, kind="stable")
        counts = np.bincount(core_of, minlength=N_CORES)
        offs_pre = np.concatenate([[0], np.cumsum(counts)])
        node_lo = np.arange(N_CORES, dtype=np.int64) * node_span
    e_pad = max(GSZ, int(math.ceil(counts.max() / GSZ)) * GSZ)

    ng_ = e_pad // GSZ
    nwb_ = u_pad // 512
    # per-chunk: how many 512-node x-write batches the gather depends on
    gdeps = []
    for c in range(ng_):
        mx = 0
        for cc in range(N_CORES):
            ids = order[offs_pre[cc] + c * GSZ:
                        min(offs_pre[cc] + (c + 1) * GSZ, offs_pre[cc + 1])]
            if ids.size:
                mx = max(mx, int((base[ids] - node_lo[cc]).max()))
        gdeps.append(min(nwb_, (mx + M + 511) // 512))
    gdeps = tuple(gdeps)
    key = (u_pad, e_pad, gdeps)
    if key not in _GRAPH_CACHE:
        _GRAPH_CACHE[key] = _build_graph(u_pad, e_pad, gdeps)
    nc = _GRAPH_CACHE[key]

    shared = _fold_weights(W_enc, b_enc, gn_gamma, gn_beta, gn_alpha, cheb_W,
                           cheb_b, lin_W, lin_b)
    bf16 = _bf16_dtype()
    nb = u_pad // 512
    ns16 = e_pad // 16

    in_maps = []
    offs = offs_pre
    for c in range(N_CORES):
        lo = int(node_lo[c])
        sl = pos_set[lo:min(lo + u_pad, N)]
        if sl.shape[0] < u_pad:
            sl = np.concatenate(
                [sl, np.zeros((u_pad - sl.shape[0], F), np.float32)], 0)
        # posT[b, f, k, n'] = sl[512 b + n', 128 k + f]
        posT = np.ascontiguousarray(
            sl.reshape(nb, 512, 4, 128).transpose(0, 3, 2, 1)).astype(bf16)
        ids = order[offs[c]:offs[c + 1]]
        loc = (base[ids] - lo).astype(np.int64)
        idx = np.zeros(e_pad, np.int16)
        idx[:loc.size] = loc.astype(np.int16)
        # wrapped layout: element i lives at [i % 16, i // 16], replicated
        # across the eight 16-partition groups
        w16 = idx.reshape(ns16, 16).T           # [16, ns16]
        m = dict(shared)
        m["posT"] = posT
        m["idx"] = np.ascontiguousarray(np.tile(w16, (8, 1)))
        in_maps.append(m)

    import os

    from concourse.bass_utils import run_bass_kernel_spmd

    trace = bool(os.environ.get("CHESHIRE_TRACE"))
    res = run_bass_kernel_spmd(nc, in_maps, core_ids=list(range(N_CORES)),
                               trace=trace)
    global LAST_EXEC_NS, LAST_RESULT
    LAST_EXEC_NS = res.exec_time_ns
    LAST_RESULT = res
    w12 = np.asarray(lin_W, np.float32).reshape(2, 128)  # [2, 128]
    lb = float(np.asarray(lin_b).reshape(-1)[0])
    out_full = np.zeros(E, np.float32)
    for c in range(N_CORES):
        ids = order[offs[c]:offs[c + 1]]
        ymm = np.asarray(res.results[c]["ymm"], np.float32)   # [128, e_pad]
        rms = np.asarray(res.results[c]["rms"], np.float32)
        logits = w12[0] @ ymm + w12[1] @ rms + lb             # [e_pad]
        vals = 1.0 / (1.0 + np.exp(-logits[:ids.size]))
        out_full[ids] = vals
    return out_full


# revision 56
# speedup vs baseline: 1.0197x; 1.0197x over previous
"""CHESHIRE hyperedge link predictor on 8 Trainium2 NeuronCores.

Structure exploited (verified at runtime):
  - members[e] = base[e] + arange(8): each hyperedge is a contiguous
    8-node window -> sorting hyperedges by base makes the whole problem
    embarrassingly parallel across a node-range partition (no collectives).
  - edge_index is the full directed 8-clique per hyperedge -> deg == 7,
    w == -1/7, so the K=3 ChebConv folds into a per-entry GEMM plus a
    per-window GEMM:  u_i = (r * x_i) @ Wap + (r * S) @ Wd + D0
    with r = 1/sqrt(var+eps) (GraphNorm fold), S = window sum of x.
  - clip commutes with max/min pools; clip(u)^2 == min(u^2, 1) for the rms
    pool, so hardtanh is applied to pooled maxima and to u^2.

Implementation notes (v3):
  - pos / W_enc cast to bf16 on host (halves the HBM stream; full-rate PE).
  - encoder channel-major (512-wide matmuls), PE transposes to rows, DRAM
    roundtrip for the SWDGE transpose-gather (as v1).
  - hardtanh applied to the *gathered* xe (DVE tensor_scalar 4x mode)
    instead of per-node, freeing phase A.
  - all phase-B elementwise bf16 packed SBUF (DVE 2x), strided mega-ops.
  - Q = sum(x^2) and ssq = sum(min(u^2,1)) on the PE (identity-matmul
    accumulation); D0 folded into the ACT psum evacuation bias.
  - only Sqrt/Square/Identity ACT functions (single table set); sigmoid on
    host; 1/sqrt via DVE reciprocal_approx_fast (accuracy).
  - GraphNorm variance chain on GPSIMD (idle otherwise).
"""

import math

import numpy as np

N_CORES = 8
M = 8          # nodes per hyperedge
D = 128        # embedding dim
F = 512        # input feature dim
EPS = 1e-5
GSZ = 512      # windows per chunk (one f32 PSUM bank per entry)

_GRAPH_CACHE = {}
LAST_EXEC_NS = None
LAST_RESULT = None


def _bf16_dtype():
    import ml_dtypes

    return np.dtype(ml_dtypes.bfloat16)


def _fold_weights(W_enc, b_enc, gn_gamma, gn_beta, gn_alpha, cheb_W, cheb_b,
                  lin_W, lin_b):
    f32 = np.float32
    W0, W1, W2 = (np.asarray(cheb_W[i], f32) for i in range(3))
    gam = np.asarray(gn_gamma, f32)
    bet = np.asarray(gn_beta, f32)
    alp = np.asarray(gn_alpha, f32)
    Wa = W0 + W1 / f32(7.0) - f32(47.0 / 49.0) * W2
    Wb = -W1 / f32(7.0) + f32(12.0 / 49.0) * W2
    Wap = gam[:, None] * Wa
    Wd = -(gam * alp / f32(8.0))[:, None] * (Wa + f32(8.0) * Wb) + gam[:, None] * Wb
    D0 = bet @ Wa + f32(8.0) * (bet @ Wb) + np.asarray(cheb_b, f32)
    neg_ka8 = -(f32(2.0) * alp - alp * alp) / f32(8.0)
    bf16 = _bf16_dtype()
    return {
        "wenc": np.ascontiguousarray(
            np.asarray(W_enc, f32).reshape(4, 128, 128)
            .transpose(1, 0, 2)).astype(bf16),
        "wap": np.ascontiguousarray(Wap.astype(bf16)),
        "wd": np.ascontiguousarray(Wd.astype(bf16)),
        "d0": np.ascontiguousarray(D0.reshape(128, 1)),
        "nka8": np.ascontiguousarray(neg_ka8.reshape(128, 1)),
        "ident": np.eye(128, dtype=bf16),
    }


def _build_graph(u_pad, e_pad, gdeps=None):
    """Build the per-core Bass graph. SPMD: same graph on all 8 cores."""
    import concourse.bass as bass
    import concourse.tile as tile
    from concourse import bacc, mybir

    f32 = mybir.dt.float32
    bf16 = mybir.dt.bfloat16
    i16 = mybir.dt.int16
    AF = mybir.ActivationFunctionType
    OP = mybir.AluOpType

    nb = u_pad // 512          # encoder blocks (13)
    ng = e_pad // GSZ          # window chunks
    ns16 = e_pad // 16

    nc = bacc.Bacc()
    # posT[b, f, k, n'] = pos[512 b + n', 128 k + f]   (rhs tiles, bf16)
    posT_p = nc.declare_dram_parameter("posT", [nb, 128, 4, 512], bf16, False)
    idx_p = nc.declare_dram_parameter("idx", [128, ns16], i16, False)
    wenc_p = nc.declare_dram_parameter("wenc", [128, 4, 128], bf16, False)
    wap_p = nc.declare_dram_parameter("wap", [128, 128], bf16, False)
    wd_p = nc.declare_dram_parameter("wd", [128, 128], bf16, False)
    d0_p = nc.declare_dram_parameter("d0", [128, 1], f32, False)
    nka8_p = nc.declare_dram_parameter("nka8", [128, 1], f32, False)
    ident_p = nc.declare_dram_parameter("ident", [128, 128], bf16, False)
    # pooled outputs: ymm = clip(max)-clip(min), rms^2*8 = ssq; the tiny
    # [2C -> 1] logits GEMM + sigmoid run on the host
    ymm_p = nc.declare_dram_parameter("ymm", [128, e_pad], bf16, True)
    rms_p = nc.declare_dram_parameter("rms", [128, e_pad], bf16, True)

    # x rows, bf16, padded so overlapping window reads stay in bounds
    x_dram = nc.dram_tensor("x_scratch", [u_pad + M, 128], bf16)

    with tile.TileContext(nc) as tc:
        with (
            tc.tile_pool(name="consts", bufs=1) as consts,
            tc.tile_pool(name="psum_u", bufs=2, space="PSUM") as psum_u,
            tc.tile_pool(name="psum_aux", bufs=3, space="PSUM") as psum_aux,
            tc.tile_pool(name="psum_tr", bufs=1, space="PSUM") as psum_tr,
        ):
            # ---- constants ----
            # wenc (needed by the first matmul) goes first on the scalar
            # queue; everything else rides the otherwise-idle vector/gpsimd
            # queues so the head stays short
            wenc_t = consts.tile([128, 4, 128], bf16)
            nc.scalar.dma_start(out=wenc_t[:, :, :], in_=wenc_p[:, :, :])
            ident_t = consts.tile([128, 128], bf16)
            nc.scalar.dma_start(out=ident_t[:, :], in_=ident_p[:, :])
            wap_t = consts.tile([128, 128], bf16)
            nc.gpsimd.dma_start(out=wap_t[:, :], in_=wap_p[:, :])
            wd_t = consts.tile([128, 128], bf16)
            nc.gpsimd.dma_start(out=wd_t[:, :], in_=wd_p[:, :])
            d0_t = consts.tile([128, 1], f32)
            nc.gpsimd.dma_start(out=d0_t[:, :], in_=d0_p[:, :])
            nka8_t = consts.tile([128, 1], f32)
            nc.gpsimd.dma_start(out=nka8_t[:, :], in_=nka8_p[:, :])
            idx_t = consts.tile([128, ns16], i16)
            nc.gpsimd.dma_start(out=idx_t[:, :], in_=idx_p[:, :])
            eps_t = consts.tile([128, 1], f32)
            nc.vector.memset(eps_t[:, :], EPS)
            from concourse import library_config
            nc.gpsimd.load_library(library_config.mlp)

            # ---- phase A: x rows = hardtanh(pos @ W_enc) ----
            # two passes so the PE sees long uninterrupted matmul streams
            # (p-state ramps to full clock): (1) all encoder GEMM blocks,
            # (2) all transposes.  The hardtanh is folded into the DVE
            # psum evacuation.
            x_writes = []
            with (
                tc.tile_pool(name="pos", bufs=3) as pos_pool,
                tc.tile_pool(name="stage", bufs=3) as stage_pool,
            ):
                xT_all = consts.tile([128, u_pad], bf16)
                for b in range(nb):
                    pos_tile = pos_pool.tile([128, 4, 512], bf16, tag="pos")
                    nc.sync.dma_start(out=pos_tile[:, :, :],
                                      in_=posT_p[b, :, :, :])
                    # encoder borrows the (idle) u-GEMM psum ring
                    psu = psum_u.tile([128, 2, GSZ], f32, tag="u")
                    ps = psu[:, 0, :]
                    for k in range(4):
                        nc.tensor.matmul(
                            ps,
                            lhsT=wenc_t[:, k, :],
                            rhs=pos_tile[:, k, :],
                            start=(k == 0),
                            stop=(k == 3),
                        )
                    nc.vector.tensor_scalar(
                        out=xT_all[:, b * 512:(b + 1) * 512], in0=ps,
                        scalar1=1.0, scalar2=-1.0, op0=OP.min, op1=OP.max)
                for b in range(nb):
                    # two alternating half-bank tiles so consecutive blocks
                    # of transposes pipeline against the stage copies
                    pst = psum_tr.tile([128, 4, 128], bf16, tag="tr")
                    for j in range(4):
                        nc.tensor.transpose(
                            out=pst[:, j, :],
                            in_=xT_all[:, b * 512 + j * 128:
                                       b * 512 + (j + 1) * 128],
                            identity=ident_t[:, :],
                        )
                    stg = stage_pool.tile([128, 4, 128], bf16, tag="stage")
                    nc.vector.tensor_copy(out=stg[:, :, :], in_=pst[:, :, :])
                    out_ap = bass.AP(
                        tensor=x_dram, offset=b * 512 * 128,
                        ap=[[128, 128], [128 * 128, 4], [1, 128]])
                    # scalar queue: the sync queue is saturated with pos
                    # loads, and gathers block on these writes
                    w = nc.scalar.dma_start(out=out_ap, in_=stg[:, :, :])
                    x_writes.append(w)
                zr = stage_pool.tile([128, 128], bf16, tag="zpad")
                nc.vector.memset(zr[:, :], 0)
                w = nc.gpsimd.dma_start(out=x_dram[u_pad:u_pad + M, :],
                                        in_=zr[0:M, :])
                x_writes.append(w)

            with (
                tc.tile_pool(name="xe", bufs=4) as xe_pool,
                tc.tile_pool(name="work", bufs=3) as work_pool,
                tc.tile_pool(name="stats", bufs=2) as stats_pool,
            ):
                # ---- phase B: gather windows (transposed), per chunk ----
                nsc = GSZ // 16
                x_view = bass.AP(tensor=x_dram, offset=0,
                                 ap=[[128, u_pad], [1, M * 128]])
                xes = []
                for c in range(ng):
                    xeT = xe_pool.tile([128, M, GSZ], bf16, tag="xe")
                    xes.append(xeT)
                    g = nc.gpsimd.dma_gather(
                        out_ap=xeT[:, :, :],
                        in_ap=x_view,
                        idxs_ap=idx_t[:, c * nsc:(c + 1) * nsc],
                        num_idxs=GSZ,
                        num_idxs_reg=GSZ,
                        elem_size=M * 128,
                        elem_step=128,
                        transpose=True,
                    )
                    if gdeps is None:
                        deps = x_writes
                    else:
                        deps = x_writes[:gdeps[c]] + [x_writes[-1]]
                    for w in deps:
                        tile.add_dep_helper(g.ins, w.ins, reason="x_dram RAW")

                ymm_all = consts.tile([128, e_pad], bf16)
                rms_all = consts.tile([128, e_pad], bf16)
                for c in range(ng):
                    cs = slice(c * GSZ, (c + 1) * GSZ)
                    xeT = xes[c]
                    # psum ring slot for ssq reserved FIRST: it is freed
                    # late (rms evac) and used late, while Q/S slots free
                    # early -- this alignment keeps chunk c+1's early
                    # matmuls off chunk c's critical tail
                    sps = psum_aux.tile([128, GSZ], f32, tag="aux")
                    # S = sum_e x on PE first -- it depends only on the
                    # gather, so the variance chain starts before x^2/Q.
                    # (Exact f32 accumulation: the v = Q + nka8*S^2
                    # cancellation needs S/Q errors to correlate, so both
                    # come from f32 sums of the same bf16 entries.)
                    ssp = psum_aux.tile([128, GSZ], f32, tag="aux")
                    for e in range(M):
                        nc.tensor.matmul(ssp[0:128, :], lhsT=ident_t[:, :],
                                         rhs=xeT[:, e, :], start=(e == 0),
                                         stop=(e == M - 1))
                    S = stats_pool.tile([128, GSZ], f32, tag="S")
                    nc.scalar.activation(S[:, :], ssp[0:128, :], AF.Identity)
                    # x^2: half on ACT, half on DVE
                    sq = work_pool.tile([128, M, GSZ], bf16, tag="sq")
                    nc.scalar.activation(sq[:, 0:4, :], xeT[:, 0:4, :],
                                         AF.Square)
                    nc.vector.tensor_tensor(
                        out=sq[:, 4:8, :], in0=xeT[:, 4:8, :],
                        in1=xeT[:, 4:8, :], op=OP.mult)
                    # Q = sum_e x^2 on PE
                    qps = psum_aux.tile([128, GSZ], f32, tag="aux")
                    for e in range(M):
                        nc.tensor.matmul(qps[:, :], lhsT=ident_t[:, :],
                                         rhs=sq[:, e, :], start=(e == 0),
                                         stop=(e == M - 1))
                    Qsb = stats_pool.tile([128, GSZ], f32, tag="Q")
                    nc.scalar.activation(Qsb[:, :], qps[:, :], AF.Identity)
                    # v = Q + nka8 * S^2 (STT on DVE, add on GPSIMD)
                    v = stats_pool.tile([128, GSZ], f32, tag="v")
                    nc.vector.scalar_tensor_tensor(
                        out=v[:, :], in0=S[:, :], scalar=nka8_t[:, 0:1],
                        in1=S[:, :], op0=OP.mult, op1=OP.mult)
                    nc.vector.tensor_tensor(
                        out=v[:, :], in0=v[:, :], in1=Qsb[:, :], op=OP.add)
                    # r = 1/sqrt(v/8 + eps): ACT Sqrt + DVE fast reciprocal
                    sv = stats_pool.tile([128, GSZ], f32, tag="sv")
                    nc.scalar.activation(sv[:, :], v[:, :], AF.Sqrt,
                                         bias=eps_t[:, 0:1], scale=0.125)
                    rf = stats_pool.tile([128, GSZ], f32, tag="rf")
                    nc.vector.reciprocal_approx_fast(out=rf[:, :],
                                                     in_=sv[:, :])
                    r = stats_pool.tile([128, GSZ], bf16, tag="r")
                    nc.vector.tensor_copy(out=r[:, :], in_=rf[:, :])
                    p = stats_pool.tile([128, GSZ], bf16, tag="p")
                    nc.vector.tensor_tensor(out=p[:, :], in0=r[:, :],
                                            in1=S[:, :], op=OP.mult)
                    # xs = x * r (broadcast over entries)
                    xs = work_pool.tile([128, M, GSZ], bf16, tag="xs")
                    for e in range(M):
                        nc.vector.tensor_tensor(
                            out=xs[:, e, :], in0=xeT[:, e, :],
                            in1=r[:, :], op=OP.mult)
                    # u GEMM in four groups of 2 entries (PE/ACT pipelined);
                    # D0 folded into the ACT evacuation bias.  sq2 for the
                    # first two groups comes from a second ACT psum read
                    # (Square with the same bias); the last two from DVE.
                    u = work_pool.tile([128, M, GSZ], bf16, tag="u")
                    sq2 = work_pool.tile([128, M, GSZ], bf16, tag="sq2")
                    for g2 in range(4):
                        ups = psum_u.tile([128, 2, GSZ], f32, tag="u")
                        for j in range(2):
                            e = 2 * g2 + j
                            nc.tensor.matmul(ups[:, j, :], lhsT=wap_t[:, :],
                                             rhs=xs[:, e, :], start=True,
                                             stop=False)
                        for j in range(2):
                            nc.tensor.matmul(ups[:, j, :], lhsT=wd_t[:, :],
                                             rhs=p[:, :], start=False,
                                             stop=True)
                        es = slice(2 * g2, 2 * g2 + 2)
                        nc.scalar.activation(u[:, es, :], ups[:, :, :],
                                             AF.Identity, bias=d0_t[:, 0:1],
                                             scale=1.0)
                        if g2 < 2:
                            nc.scalar.activation(sq2[:, es, :], ups[:, :, :],
                                                 AF.Square,
                                                 bias=d0_t[:, 0:1],
                                                 scale=1.0)
                    nc.vector.tensor_tensor(
                        out=sq2[:, 4:8, :], in0=u[:, 4:8, :],
                        in1=u[:, 4:8, :], op=OP.mult)
                    # pools: umax/umin trees (DVE), clip after pooling
                    t1 = work_pool.tile([128, 4, GSZ], bf16, tag="t1")
                    nc.vector.tensor_tensor(
                        out=t1[:, :, :], in0=u[:, 0:4, :], in1=u[:, 4:8, :],
                        op=OP.max)
                    nc.vector.tensor_tensor(
                        out=t1[:, 0:2, :], in0=t1[:, 0:2, :],
                        in1=t1[:, 2:4, :], op=OP.max)
                    umax = stats_pool.tile([128, GSZ], bf16, tag="ux")
                    nc.vector.tensor_tensor(
                        out=umax[:, :], in0=t1[:, 0, :], in1=t1[:, 1, :],
                        op=OP.max)
                    nc.vector.tensor_tensor(
                        out=t1[:, :, :], in0=u[:, 0:4, :], in1=u[:, 4:8, :],
                        op=OP.min)
                    nc.vector.tensor_tensor(
                        out=t1[:, 0:2, :], in0=t1[:, 0:2, :],
                        in1=t1[:, 2:4, :], op=OP.min)
                    umin = stats_pool.tile([128, GSZ], bf16, tag="un")
                    nc.vector.tensor_tensor(
                        out=umin[:, :], in0=t1[:, 0, :], in1=t1[:, 1, :],
                        op=OP.min)
                    nc.vector.tensor_scalar(out=umax[:, :], in0=umax[:, :],
                                            scalar1=1.0, scalar2=-1.0,
                                            op0=OP.min, op1=OP.max)
                    nc.vector.tensor_scalar(out=umin[:, :], in0=umin[:, :],
                                            scalar1=1.0, scalar2=-1.0,
                                            op0=OP.min, op1=OP.max)
                    nc.vector.tensor_tensor(out=ymm_all[:, cs],
                                            in0=umax[:, :],
                                            in1=umin[:, :], op=OP.subtract)
                    # ssq = sum_e min(sq2, 1) on PE
                    nc.vector.tensor_scalar(out=sq2[:, :, :],
                                            in0=sq2[:, :, :],
                                            scalar1=1.0, scalar2=None,
                                            op0=OP.min)
                    for e in range(M):
                        nc.tensor.matmul(sps[:, :], lhsT=ident_t[:, :],
                                         rhs=sq2[:, e, :], start=(e == 0),
                                         stop=(e == M - 1))
                    nc.scalar.activation(rms_all[:, cs], sps[:, :], AF.Sqrt,
                                         scale=0.125)
                nc.gpsimd.dma_start(out=ymm_p[:, :], in_=ymm_all[:, :])
                nc.gpsimd.dma_start(out=rms_p[:, :], in_=rms_all[:, :])

    nc.finalize()
    return nc


def _np_fallback(pos_set, W_enc, b_enc, gn_gamma, gn_beta, gn_alpha, cheb_W,
                 cheb_b, lin_W, lin_b, members, edge_index, batch):
    """Pure-numpy general path (only used if the expected input structure is
    absent; inputs from setup_inputs always take the device path)."""
    f32 = np.float32
    E = members.shape[0]
    num_entries = members.size
    x = np.clip(pos_set @ W_enc + b_enc, -1.0, 1.0).astype(f32)
    xe = x[members.reshape(-1)]
    cnt = np.zeros(E, f32)
    np.add.at(cnt, batch, 1.0)
    mean = np.zeros((E, x.shape[1]), f32)
    np.add.at(mean, batch, xe)
    mean /= cnt[:, None]
    ctr = xe - gn_alpha * mean[batch]
    var = np.zeros((E, x.shape[1]), f32)
    np.add.at(var, batch, ctr * ctr)
    var /= cnt[:, None]
    xe = gn_gamma * ctr / np.sqrt(var + EPS)[batch] + gn_beta
    src, dst = edge_index[0], edge_index[1]
    deg = np.zeros(num_entries, f32)
    np.add.at(deg, dst, 1.0)
    w = -1.0 / np.sqrt(deg[src] * deg[dst])

    def lhat(h):
        out = np.zeros_like(h)
        np.add.at(out, dst, w[:, None] * h[src])
        return out

    tx0 = xe
    tx1 = lhat(tx0)
    out = tx0 @ cheb_W[0] + tx1 @ cheb_W[1]
    tkm1, tkm2 = tx1, tx0
    for k in range(2, cheb_W.shape[0]):
        tk = 2.0 * lhat(tkm1) - tkm2
        out = out + tk @ cheb_W[k]
        tkm1, tkm2 = tk, tkm1
    h = np.clip(out + cheb_b, -1.0, 1.0)
    ymax = np.full((E, h.shape[1]), -np.inf, f32)
    ymin = np.full((E, h.shape[1]), np.inf, f32)
    np.maximum.at(ymax, batch, h)
    np.minimum.at(ymin, batch, h)
    ynorm = np.zeros((E, h.shape[1]), f32)
    np.add.at(ynorm, batch, h * h)
    ynorm = np.sqrt(ynorm / cnt[:, None])
    y = np.concatenate([ymax - ymin, ynorm], axis=1)
    logits = y @ lin_W + lin_b
    return (1.0 / (1.0 + np.exp(-logits))).squeeze(-1).astype(f32)


def _has_window_structure(members, edge_index, batch):
    E, Mm = members.shape
    if Mm != M:
        return False
    base = members[:, 0]
    if not (members == base[:, None] + np.arange(M, dtype=members.dtype)).all():
        return False
    if not (batch == np.repeat(np.arange(E, dtype=batch.dtype), M)).all():
        return False
    row, col = np.where(~np.eye(M, dtype=bool))
    offs = np.arange(E, dtype=np.int64)[:, None] * M
    ei = np.stack([(offs + row[None, :]).ravel(), (offs + col[None, :]).ravel()])
    return (edge_index == ei).all()


def kernel(pos_set, W_enc, b_enc, gn_gamma, gn_beta, gn_alpha, cheb_W, cheb_b,
           lin_W, lin_b, members, edge_index, batch):
    pos_set = np.asarray(pos_set, np.float32)
    members = np.asarray(members)
    edge_index = np.asarray(edge_index)
    batch = np.asarray(batch)
    benc_zero = bool(np.all(np.asarray(b_enc) == 0.0))
    if not _has_window_structure(members, edge_index, batch) or not benc_zero:
        return _np_fallback(
            pos_set, np.asarray(W_enc, np.float32),
            np.asarray(b_enc, np.float32), np.asarray(gn_gamma, np.float32),
            np.asarray(gn_beta, np.float32), np.asarray(gn_alpha, np.float32),
            np.asarray(cheb_W, np.float32), np.asarray(cheb_b, np.float32),
            np.asarray(lin_W, np.float32), np.asarray(lin_b, np.float32),
            members, edge_index, batch)

    N = pos_set.shape[0]
    E = members.shape[0]
    base = members[:, 0].astype(np.int64)
    node_span = (N + N_CORES - 1) // N_CORES                # 6250
    u_pad = ((node_span + M + 511) // 512 + 1) * 512        # 6656 for N=50000
    # quantile split: sort windows by base, give each core an equal count.
    order = np.argsort(base, kind="stable")
    ecnt = (E + N_CORES - 1) // N_CORES
    counts = np.array([min(ecnt, E - c * ecnt) for c in range(N_CORES)])
    offs_pre = np.concatenate([[0], np.cumsum(counts)])
    node_lo = np.zeros(N_CORES, np.int64)
    ok = True
    for c in range(N_CORES):
        ids = order[offs_pre[c]:offs_pre[c + 1]]
        if ids.size == 0:
            node_lo[c] = 0
            continue
        node_lo[c] = base[ids[0]]
        if base[ids[-1]] + M - node_lo[c] > u_pad:
            ok = False
            break
    if not ok:
        core_of = np.minimum(base // node_span, N_CORES - 1)
        order = np.argsort(base, kind="stable")
        counts = np.bincount(core_of, minlength=N_CORES)
        offs_pre = np.concatenate([[0], np.cumsum(counts)])
        node_lo = np.arange(N_CORES, dtype=np.int64) * node_span
    e_pad = max(GSZ, int(math.ceil(counts.max() / GSZ)) * GSZ)

    ng_ = e_pad // GSZ
    nwb_ = u_pad // 512
    # per-chunk: how many 512-node x-write batches the gather depends on
    gdeps = []
    for c in range(ng_):
        mx = 0
        for cc in range(N_CORES):
            ids = order[offs_pre[cc] + c * GSZ:
                        min(offs_pre[cc] + (c + 1) * GSZ, offs_pre[cc + 1])]
            if ids.size:
                mx = max(mx, int((base[ids] - node_lo[cc]).max()))
        gdeps.append(min(nwb_, (mx + M + 511) // 512))
    gdeps = tuple(gdeps)
    key = (u_pad, e_pad, gdeps)
    if key not in _GRAPH_CACHE:
        _GRAPH_CACHE[key] = _build_graph(u_pad, e_pad, gdeps)
    nc = _GRAPH_CACHE[key]

    shared = _fold_weights(W_enc, b_enc, gn_gamma, gn_beta, gn_alpha, cheb_W,
                           cheb_b, lin_W, lin_b)
    bf16 = _bf16_dtype()
    nb = u_pad // 512
    ns16 = e_pad // 16

    in_maps = []
    offs = offs_pre
    for c in range(N_CORES):
        lo = int(node_lo[c])
        sl = pos_set[lo:min(lo + u_pad, N)]
        if sl.shape[0] < u_pad:
            sl = np.concatenate(
                [sl, np.zeros((u_pad - sl.shape[0], F), np.float32)], 0)
        # posT[b, f, k, n'] = sl[512 b + n', 128 k + f]
        posT = np.ascontiguousarray(
            sl.reshape(nb, 512, 4, 128).transpose(0, 3, 2, 1)).astype(bf16)
        ids = order[offs[c]:offs[c + 1]]
        loc = (base[ids] - lo).astype(np.int64)
        idx = np.zeros(e_pad, np.int16)
        idx[:loc.size] = loc.astype(np.int16)
        # wrapped layout: element i lives at [i % 16, i // 16], replicated
        # across the eight 16-partition groups
        w16 = idx.reshape(ns16, 16).T           # [16, ns16]
        m = dict(shared)
        m["posT"] = posT
        m["idx"] = np.ascontiguousarray(np.tile(w16, (8, 1)))
        in_maps.append(m)

    import os

    from concourse.bass_utils import run_bass_kernel_spmd

    trace = bool(os.environ.get("CHESHIRE_TRACE"))
    res = run_bass_kernel_spmd(nc, in_maps, core_ids=list(range(N_CORES)),
                               trace=trace)
    global LAST_EXEC_NS, LAST_RESULT
    LAST_EXEC_NS = res.exec_time_ns
    LAST_RESULT = res
    w12 = np.asarray(lin_W, np.float32).reshape(2, 128)  # [2, 128]
    lb = float(np.asarray(lin_b).reshape(-1)[0])
    out_full = np.zeros(E, np.float32)
    for c in range(N_CORES):
        ids = order[offs[c]:offs[c + 1]]
        ymm = np.asarray(res.results[c]["ymm"], np.float32)   # [128, e_pad]
        rms = np.asarray(res.results[c]["rms"], np.float32)
        logits = w12[0] @ ymm + w12[1] @ rms + lb             # [e_pad]
        vals = 1.0 / (1.0 + np.exp(-logits[:ids.size]))
        out_full[ids] = vals
    return out_full
